# revision 24
# baseline (speedup 1.0000x reference)
"""Trainium2 kernel for AutoPatchOverLapModel3D (3D patch overlap-add / fold).

Math: out[b,p,y0,y1,y2] = (1/CM[y0,y1,y2]) * sum_{j0,j1,j2}
        x[b, y0-j0, y1-j1, (y2-j2)%64, p, j0, j1, j2]
i.e. a stride-1 overlap-add of 5x5x5 patches; axes 0/1 zero-padded,
axis 2 circular; CM is the separable patch-count normalizer.

Strategy (8 NeuronCores, SPMD) — memory-roofline oriented:
  - Host casts x to bf16 (RNE; tolerance is 2e-2, bf16 costs ~2e-3) and
    permutes each 2500-vec patch to (j2, j0, j1, p) so every j2 tap is a
    contiguous 500-elem slice. HBM read per core: 22.4 MB instead of 44.8.
  - Shard 5 half-planes (70 columns = 4480 patch rows) per core.
  - Per 128-patch group (2 columns): fold the circular j2 axis with 5
    TensorE matmuls (block-diag 0/1 shift weights, bf16, PSUM f32).
  - Fold j1 AND j0 on-device with ONE 4D-AP DVE tensor_add per group into
    a persistent accumulator acc[(u,y2), y1', k, p] (k = frame-pair-local
    y0, y1' = 36 = two 18-wide half-plane windows). The u=1 column's +1
    y1 offset is absorbed into the layout and undone on the host, so a
    single 128-partition add folds both columns.
  - acc y1'-columns are flushed to DRAM as soon as no later frame can
    write them (s=1 block after frame 3; s=0 in two pieces during/after
    frame 4), so output DMA hides under input loads and the tail is one
    0.86 MB transfer.
  - Host: place per-core (k, s, u) cells at (y0, y1) (core-parity mapping
    at _stitch), divide by the counting matrix.
"""

import numpy as np

B, X0, X1, X2, P = 2, 10, 28, 64, 20
PK = 5  # patch edge
Y0, Y1, Y2 = 14, 32, 64
NCORES = 8
NCOL = B * X0 * X1                   # 560 (b,i0,i1) columns
COLS_PER_CORE = NCOL // NCORES       # 70
ROWS_PER_CORE = COLS_PER_CORE * X2   # 4480
PATCH_VEC = P * PK * PK * PK         # 2500
FREE = PK * PK * P                   # 500 per j2 tap, laid out (j0, j1, p)
GROUPS = ROWS_PER_CORE // 128        # 35 groups of 128 patches (2 cols)
GROUPS_PER_FRAME = 7                 # 14 columns = one half-plane frame
FRAMES = 5
KSPAN = 7                            # frame-local y0 span: 3 i0 values + 4
Y1SPAN = 36                          # two 18-wide half-plane y1 windows
ACC_FREE = KSPAN * Y1SPAN * P        # 5040

_CACHE = {}


def _shift_weights():
    # w[k, j2*128 + m]: k = u*64 + i2, m = u*64 + y2 ;  1.0 iff same u
    # and y2 == (i2 + j2 - 2) % 64 (circular axis keeps patch centers at
    # their own index: tap j2 lands at offset j2-2). Block-diagonal over
    # the 2 columns sharing a matmul group.
    w = np.zeros((128, 5, 128), np.float32)
    i2 = np.arange(64)
    for j2 in range(5):
        y2 = (i2 + j2 - 2) % 64
        for u in range(2):
            w[u * 64 + i2, j2, u * 64 + y2] = 1.0
    return w.reshape(128, 5 * 128)


def _kernel_body(tc, xs, w, out):
    import concourse.mybir as mybir

    nc = tc.nc
    f32 = mybir.dt.float32
    KP = KSPAN * P  # 140: free stride of one y1' column
    with (
        tc.tile_pool(name="wpool", bufs=1) as wpool,
        tc.tile_pool(name="xpool", bufs=12) as xpool,
        tc.tile_pool(name="accpool", bufs=1) as accpool,
        tc.tile_pool(name="stpool", bufs=2) as stpool,
        tc.tile_pool(name="pspool", bufs=8, space="PSUM") as pspool,
    ):
        wt = wpool.tile([128, 5 * 128], xs.dtype)
        nc.sync.dma_start(out=wt[:, :], in_=w[:, :])
        acc = accpool.tile([128, ACC_FREE], f32)
        nc.gpsimd.memset(acc[:, :], 0.0)
        av = acc[:, :].rearrange("a (y k p) -> a y k p", y=Y1SPAN, k=KSPAN)

        def flush(c0, c1):
            # columns [c0, c1) of acc are final: bf16-convert on the idle
            # Act engine (halves output bus bytes and the tail transfer),
            # then stream out from staging via the Act HWDGE queue.
            n = (c1 - c0) * KP
            st = stpool.tile([128, 10 * KP], mybir.dt.bfloat16)
            nc.scalar.copy(st[:, :n], acc[:, c0 * KP:c1 * KP])
            nc.scalar.dma_start(out=out[:, c0 * KP:c1 * KP], in_=st[:, :n])

        for g in range(GROUPS):
            h, q = divmod(g, GROUPS_PER_FRAME)
            k0, s = divmod(h, 2)
            xt = xpool.tile([128, PATCH_VEC], xs.dtype)
            nc.sync.dma_start(out=xt[:, :], in_=xs[g * 128:(g + 1) * 128, :])
            ps = pspool.tile([128, FREE], f32)
            for j2 in range(5):
                nc.tensor.matmul(
                    ps[:, :],
                    wt[:, j2 * 128:(j2 + 1) * 128],
                    xt[:, j2 * FREE:(j2 + 1) * FREE],
                    start=(j2 == 0),
                    stop=(j2 == 4),
                )
            # ps free layout (j0, j1, p) -> dst windows y1b+j1, k0+j0: one
            # 4D-AP accumulate per group. The u=1 column's extra +1 y1
            # offset is absorbed into the layout (its cells are stored
            # one slot early; the host shifts them back), so a single
            # 128-partition DVE add folds both columns at once.
            pv = ps[:, :].rearrange("a (j0 j1 p) -> a j1 j0 p", j0=PK, j1=PK)
            y1b = 18 * s + 2 * q
            dst = av[:, y1b:y1b + 5, k0:k0 + 5, :]
            nc.vector.tensor_add(dst, dst, pv[:, :, :, :])
            # flush columns as soon as no later frame can write them:
            # s=1 cols during/after frame 3, s=0 cols as frame 4 sweeps.
            # All flush DMAs ride under remaining input loads except the
            # final 6-column piece (0.2 MB bf16).
            if g == 3 * GROUPS_PER_FRAME + 3:
                flush(18, 26)
            elif g == 4 * GROUPS_PER_FRAME - 1:
                flush(26, 36)
            elif g == 4 * GROUPS_PER_FRAME + 2:
                flush(0, 6)
            elif g == 4 * GROUPS_PER_FRAME + 5:
                flush(6, 12)
        flush(12, 18)


def _build_nc():
    import concourse.bacc as bacc
    import concourse.mybir as mybir
    import concourse.tile as tile

    nc = bacc.Bacc(
        "TRN2",
        target_bir_lowering=False,
        debug=False,
        enable_asserts=True,
        num_devices=NCORES,
    )
    f32 = mybir.dt.float32
    bf16 = mybir.dt.bfloat16
    xs = nc.declare_dram_parameter(
        "xs", [ROWS_PER_CORE, PATCH_VEC], bf16, isOutput=False
    )
    w = nc.declare_dram_parameter("w", [128, 5 * 128], bf16, isOutput=False)
    out = nc.declare_dram_parameter(
        "out", [128, ACC_FREE], bf16, isOutput=True
    )

    with tile.TileContext(nc) as tc:
        _kernel_body(tc, xs, w, out)
    nc.compile()
    return nc


def _counting_matrix():
    c0 = np.zeros(Y0, np.float32)
    for i0 in range(X0):
        c0[i0:i0 + PK] += 1
    c1 = np.zeros(Y1, np.float32)
    for i1 in range(X1):
        c1[i1:i1 + PK] += 1
    return c0[:, None, None] * c1[None, :, None] * 5.0


def _make_in_maps(x):
    import ml_dtypes

    # bf16 RNE cast first (contiguous, fast), then patch-dim permute
    # (p, j0, j1, j2) -> (j2, j0, j1, p) so each j2 tap is a contiguous
    # 500-elem slice whose (j0, j1, p) order matches the accumulator.
    xb = x.reshape(NCOL * X2, P, PK, PK, PK).astype(ml_dtypes.bfloat16)
    xb = np.ascontiguousarray(xb.transpose(0, 4, 2, 3, 1)).reshape(
        NCOL * X2, PATCH_VEC
    )
    wnp = _shift_weights().astype(ml_dtypes.bfloat16)
    return [
        {"xs": xb[c * ROWS_PER_CORE:(c + 1) * ROWS_PER_CORE], "w": wnp}
        for c in range(NCORES)
    ]


def _stitch(oc):
    # oc: [c, 128, 5040] -> [c, u, y2, s, y1f, k, p].
    # Device frame h wrote (k0=h//2, s=h%2). True (i0rel, half) per core
    # parity: even cores (h//2, h%2); odd cores ((h+1)//2, (h+1)%2) — so
    # cell (k, s) is (y0 = i0a + k, half = s) on even cores and
    # (y0 = i0a + k + s, half = 1-s) on odd cores. The u=1 column's
    # cells are stored one y1 slot early (see kernel body): shift by +u.
    ocr = oc.reshape(NCORES, 2, 64, 2, 18, KSPAN, P)
    out = np.zeros((B, P, Y0, Y1, Y2), np.float32)
    for c in range(NCORES):
        g0 = (5 * c) // 2
        b, i0a = divmod(g0, X0)
        odd = c % 2
        for s in range(2):
            half = (1 - s) if odd else s
            dy0 = i0a + (s if odd else 0)
            kmax = min(KSPAN, Y0 - dy0)  # trailing cells beyond Y0 are 0
            for u in range(2):
                wid = 18 - u  # u=1's last stored slot is never written
                blk = ocr[c, u, :, s, :wid, :kmax, :]    # [y2, y1f, k, p]
                y1lo = 14 * half + u
                out[b, :, dy0:dy0 + kmax, y1lo:y1lo + wid, :] += (
                    blk.transpose(3, 2, 1, 0)
                )
    return out / _counting_matrix()


def kernel(x: np.ndarray) -> np.ndarray:
    from concourse.bass_utils import run_bass_kernel_spmd

    if "nc" not in _CACHE:
        _CACHE["nc"] = _build_nc()
    nc = _CACHE["nc"]
    in_maps = _make_in_maps(x)
    res = run_bass_kernel_spmd(nc, in_maps, list(range(NCORES)))
    oc = np.stack(
        [res.results[c]["out"] for c in range(NCORES)], axis=0
    ).astype(np.float32)
    return _stitch(oc)


# revision 25
# speedup vs baseline: 1.1574x; 1.1574x over previous
"""Trainium2 kernel for AutoPatchOverLapModel3D (3D patch overlap-add / fold).

Math: out[b,p,y0,y1,y2] = (1/CM[y0,y1,y2]) * sum_{j0,j1,j2}
        x[b, y0-j0, y1-j1, (y2-j2)%64, p, j0, j1, j2]
i.e. a stride-1 overlap-add of 5x5x5 patches; axes 0/1 zero-padded,
axis 2 circular; CM is the separable patch-count normalizer.

Strategy (8 NeuronCores, SPMD) — memory-roofline oriented:
  - Host casts x to bf16 (RNE; tolerance is 2e-2, bf16 costs ~2e-3) and
    permutes each 2500-vec patch to (j2, j0, j1, p) so every j2 tap is a
    contiguous 500-elem slice. HBM read per core: 22.4 MB instead of 44.8.
  - Shard 5 half-planes (70 columns = 4480 patch rows) per core.
  - Per 128-patch group (2 columns): fold the circular j2 axis with 5
    TensorE matmuls (block-diag 0/1 shift weights, bf16, PSUM f32).
  - Fold j1 AND j0 on-device with ONE 4D-AP DVE tensor_add per group into
    a persistent accumulator acc[(u,y2), y1', k, p] (k = frame-pair-local
    y0, y1' = 36 = two 18-wide half-plane windows). The u=1 column's +1
    y1 offset is absorbed into the layout and undone on the host, so a
    single 128-partition add folds both columns.
  - acc y1'-columns are flushed to DRAM as soon as no later frame can
    write them (s=1 block after frame 3; s=0 in two pieces during/after
    frame 4), so output DMA hides under input loads and the tail is one
    0.86 MB transfer.
  - Host: place per-core (k, s, u) cells at (y0, y1) (core-parity mapping
    at _stitch), divide by the counting matrix.
"""

import numpy as np

B, X0, X1, X2, P = 2, 10, 28, 64, 20
PK = 5  # patch edge
Y0, Y1, Y2 = 14, 32, 64
NCORES = 8
NCOL = B * X0 * X1                   # 560 (b,i0,i1) columns
COLS_PER_CORE = NCOL // NCORES       # 70
ROWS_PER_CORE = COLS_PER_CORE * X2   # 4480
PATCH_VEC = P * PK * PK * PK         # 2500
FREE = PK * PK * P                   # 500 per j2 tap, laid out (j0, j1, p)
GROUPS = ROWS_PER_CORE // 128        # 35 groups of 128 patches (2 cols)
GROUPS_PER_FRAME = 7                 # 14 columns = one half-plane frame
FRAMES = 5
KSPAN = 7                            # frame-local y0 span: 3 i0 values + 4
Y1SPAN = 36                          # two 18-wide half-plane y1 windows
ACC_FREE = KSPAN * Y1SPAN * P        # 5040

_CACHE = {}


def _shift_weights():
    # w[k, j2*128 + m]: k = u*64 + i2, m = u*64 + y2 ;  1.0 iff same u
    # and y2 == (i2 + j2 - 2) % 64 (circular axis keeps patch centers at
    # their own index: tap j2 lands at offset j2-2). Block-diagonal over
    # the 2 columns sharing a matmul group.
    w = np.zeros((128, 5, 128), np.float32)
    i2 = np.arange(64)
    for j2 in range(5):
        y2 = (i2 + j2 - 2) % 64
        for u in range(2):
            w[u * 64 + i2, j2, u * 64 + y2] = 1.0
    return w.reshape(128, 5 * 128)


def _kernel_body(tc, xs, w, out):
    import concourse.mybir as mybir

    nc = tc.nc
    f32 = mybir.dt.float32
    KP = KSPAN * P  # 140: free stride of one y1' column
    with (
        tc.tile_pool(name="wpool", bufs=1) as wpool,
        tc.tile_pool(name="xpool", bufs=12) as xpool,
        tc.tile_pool(name="accpool", bufs=1) as accpool,
        tc.tile_pool(name="stpool", bufs=2) as stpool,
        tc.tile_pool(name="pspool", bufs=8, space="PSUM") as pspool,
    ):
        wt = wpool.tile([128, 5 * 128], xs.dtype)
        nc.sync.dma_start(out=wt[:, :], in_=w[:, :])
        acc = accpool.tile([128, ACC_FREE], f32)
        nc.gpsimd.memset(acc[:, :], 0.0)
        av = acc[:, :].rearrange("a (y k p) -> a y k p", y=Y1SPAN, k=KSPAN)

        def flush(c0, c1):
            # columns [c0, c1) of acc are final: bf16-convert on the idle
            # Act engine (halves output bus bytes and the tail transfer),
            # then stream out from staging via the Act HWDGE queue.
            n = (c1 - c0) * KP
            st = stpool.tile([128, 10 * KP], mybir.dt.bfloat16)
            nc.scalar.copy(st[:, :n], acc[:, c0 * KP:c1 * KP])
            nc.scalar.dma_start(out=out[:, c0 * KP:c1 * KP], in_=st[:, :n])

        xt = None
        for g in range(GROUPS):
            h, q = divmod(g, GROUPS_PER_FRAME)
            k0, s = divmod(h, 2)
            # groups are host-interleaved in pairs by patch index, so one
            # DMA per pair moves 10 KB/partition in single descriptors
            gp, gs = divmod(g, 2)
            if gs == 0:
                n = 2 * PATCH_VEC if g < GROUPS - 1 else PATCH_VEC
                xt = xpool.tile([128, 2 * PATCH_VEC], xs.dtype)
                nc.sync.dma_start(
                    out=xt[:, :n],
                    in_=xs[gp * 256:gp * 256 + (n // PATCH_VEC) * 128, :]
                    .rearrange("(a s) f -> a (s f)", s=n // PATCH_VEC),
                )
            xv = xt[:, gs * PATCH_VEC:(gs + 1) * PATCH_VEC]
            ps = pspool.tile([128, FREE], f32)
            for j2 in range(5):
                nc.tensor.matmul(
                    ps[:, :],
                    wt[:, j2 * 128:(j2 + 1) * 128],
                    xv[:, j2 * FREE:(j2 + 1) * FREE],
                    start=(j2 == 0),
                    stop=(j2 == 4),
                )
            # ps free layout (j0, j1, p) -> dst windows y1b+j1, k0+j0: one
            # 4D-AP accumulate per group. The u=1 column's extra +1 y1
            # offset is absorbed into the layout (its cells are stored
            # one slot early; the host shifts them back), so a single
            # 128-partition DVE add folds both columns at once.
            pv = ps[:, :].rearrange("a (j0 j1 p) -> a j1 j0 p", j0=PK, j1=PK)
            y1b = 18 * s + 2 * q
            dst = av[:, y1b:y1b + 5, k0:k0 + 5, :]
            nc.vector.tensor_add(dst, dst, pv[:, :, :, :])
            # flush columns as soon as no later frame can write them:
            # s=1 cols during/after frame 3, s=0 cols as frame 4 sweeps.
            # All flush DMAs ride under remaining input loads except the
            # final 6-column piece (0.2 MB bf16).
            if g == 3 * GROUPS_PER_FRAME + 3:
                flush(18, 26)
            elif g == 4 * GROUPS_PER_FRAME - 1:
                flush(26, 36)
            elif g == 4 * GROUPS_PER_FRAME + 2:
                flush(0, 6)
            elif g == 4 * GROUPS_PER_FRAME + 5:
                flush(6, 12)
        flush(12, 18)


def _build_nc():
    import concourse.bacc as bacc
    import concourse.mybir as mybir
    import concourse.tile as tile

    nc = bacc.Bacc(
        "TRN2",
        target_bir_lowering=False,
        debug=False,
        enable_asserts=True,
        num_devices=NCORES,
    )
    f32 = mybir.dt.float32
    bf16 = mybir.dt.bfloat16
    xs = nc.declare_dram_parameter(
        "xs", [ROWS_PER_CORE, PATCH_VEC], bf16, isOutput=False
    )
    w = nc.declare_dram_parameter("w", [128, 5 * 128], bf16, isOutput=False)
    out = nc.declare_dram_parameter(
        "out", [128, ACC_FREE], bf16, isOutput=True
    )

    with tile.TileContext(nc) as tc:
        _kernel_body(tc, xs, w, out)
    nc.compile()
    return nc


def _counting_matrix():
    c0 = np.zeros(Y0, np.float32)
    for i0 in range(X0):
        c0[i0:i0 + PK] += 1
    c1 = np.zeros(Y1, np.float32)
    for i1 in range(X1):
        c1[i1:i1 + PK] += 1
    return c0[:, None, None] * c1[None, :, None] * 5.0


def _make_in_maps(x):
    import ml_dtypes

    # bf16 RNE cast first (contiguous, fast), then patch-dim permute
    # (p, j0, j1, j2) -> (j2, j0, j1, p) so each j2 tap is a contiguous
    # 500-elem slice whose (j0, j1, p) order matches the accumulator.
    xb = x.reshape(NCOL * X2, P, PK, PK, PK).astype(ml_dtypes.bfloat16)
    xb = np.ascontiguousarray(xb.transpose(0, 4, 2, 3, 1)).reshape(
        NCOL * X2, PATCH_VEC
    )
    # interleave group pairs by patch index: per core, rows become
    # [pair t][patch p][group-in-pair s][2500] so each partition's two
    # patches are DRAM-adjacent (10 KB descriptors); group 34 stays as-is
    xc = xb.reshape(NCORES, GROUPS, 128, PATCH_VEC)
    xp = np.concatenate(
        [
            xc[:, :GROUPS - 1].reshape(NCORES, (GROUPS - 1) // 2, 2, 128,
                                       PATCH_VEC).transpose(0, 1, 3, 2, 4)
            .reshape(NCORES, (GROUPS - 1) * 128, PATCH_VEC),
            xc[:, GROUPS - 1],
        ],
        axis=1,
    )
    xb = np.ascontiguousarray(xp).reshape(NCORES * GROUPS * 128, PATCH_VEC)
    wnp = _shift_weights().astype(ml_dtypes.bfloat16)
    return [
        {"xs": xb[c * ROWS_PER_CORE:(c + 1) * ROWS_PER_CORE], "w": wnp}
        for c in range(NCORES)
    ]


def _stitch(oc):
    # oc: [c, 128, 5040] -> [c, u, y2, s, y1f, k, p].
    # Device frame h wrote (k0=h//2, s=h%2). True (i0rel, half) per core
    # parity: even cores (h//2, h%2); odd cores ((h+1)//2, (h+1)%2) — so
    # cell (k, s) is (y0 = i0a + k, half = s) on even cores and
    # (y0 = i0a + k + s, half = 1-s) on odd cores. The u=1 column's
    # cells are stored one y1 slot early (see kernel body): shift by +u.
    ocr = oc.reshape(NCORES, 2, 64, 2, 18, KSPAN, P)
    out = np.zeros((B, P, Y0, Y1, Y2), np.float32)
    for c in range(NCORES):
        g0 = (5 * c) // 2
        b, i0a = divmod(g0, X0)
        odd = c % 2
        for s in range(2):
            half = (1 - s) if odd else s
            dy0 = i0a + (s if odd else 0)
            kmax = min(KSPAN, Y0 - dy0)  # trailing cells beyond Y0 are 0
            for u in range(2):
                wid = 18 - u  # u=1's last stored slot is never written
                blk = ocr[c, u, :, s, :wid, :kmax, :]    # [y2, y1f, k, p]
                y1lo = 14 * half + u
                out[b, :, dy0:dy0 + kmax, y1lo:y1lo + wid, :] += (
                    blk.transpose(3, 2, 1, 0)
                )
    return out / _counting_matrix()


def kernel(x: np.ndarray) -> np.ndarray:
    from concourse.bass_utils import run_bass_kernel_spmd

    if "nc" not in _CACHE:
        _CACHE["nc"] = _build_nc()
    nc = _CACHE["nc"]
    in_maps = _make_in_maps(x)
    res = run_bass_kernel_spmd(nc, in_maps, list(range(NCORES)))
    oc = np.stack(
        [res.results[c]["out"] for c in range(NCORES)], axis=0
    ).astype(np.float32)
    return _stitch(oc)
